# Initial kernel scaffold
#
"""Trainium2 Bass kernel for nn_InterpolationModel (NaN-gap linear interpolation).

Problem: x [256, 2048, 22, 2] f32, one contiguous NaN gap along T per batch row.
Output: x with the gap filled by linear interpolation between the last valid
frame before the gap (s) and the first valid frame after it (e).

Strategy (pure data parallel over batch, 32 rows per core):
  - Bulk copy x -> y through SBUF in 4 chunks of 8 rows ([128, 5632] tiles,
    partition = 128 consecutive frames, 22.5KB contiguous per partition).
  - While each chunk transits SBUF, sample element 0 of every frame
    (stride-44 AP) and reduce per partition: min(t + 65536*valid) and
    min((valid-1)*t) -> first/last NaN frame per partition.
  - One TensorE transpose + grouped reduce turns per-partition partials into
    per-row s (last valid before gap), e (first valid after), 1/(e-s).
  - Per-row scalars round-trip through a tiny DRAM scratch so an indirect
    gather can replicate them to a [128, .] layout (4 partitions per row).
  - Fixed 512-frame window starting at s+1 always covers the whole gap
    (gap <= 511) and never leaves the row (s < 1024 => s+513 <= 1536 < 2048).
    Gather the window [128, 5632], compute interp = xs + (t-s)*slope with
    broadcast APs, keep original values where not NaN (copy_predicated),
    scatter back over y at the same offsets.

Everything is a single Tile-scheduled program; the only cross-phase sync
needed beyond Tile's tracking is "scatter after bulk stores" and
"scalar gather after scratch write", wired with add_dep_helper.
"""

import numpy as np

import concourse.bacc as bacc
import concourse.bass as bass
import concourse.mybir as mybir
import concourse.tile as tile
from bass_rust import add_dep_helper
from concourse.masks import make_identity

F32 = mybir.dt.float32
I32 = mybir.dt.int32
AX = mybir.AxisListType
OP = mybir.AluOpType

# Full problem: B=256, T=2048, A=22, D=2 over 8 cores.
B, T, A, D = 256, 2048, 22, 2
C = A * D            # 44 contiguous f32 per frame
NCORES = 8
R = B // NCORES      # 32 rows per core
CHUNKS = 4           # bulk-copy chunks per core
RCH = R // CHUNKS    # 8 rows per chunk
P = 128
FPP = (RCH * T) // P  # 128 frames per partition in a chunk tile
PPR = T // FPP        # 16 partitions per row
WSUB = 4              # window partitions per row (4 * 32 rows = 128)
WF = 128              # frames per window partition (window = 512 frames)
BIG = 65536.0


def _bcast_mid(ap, count):
    """[P, n] AP -> [P, count, n] with a stride-0 middle axis."""
    return bass.AP(ap.tensor, ap.offset, [list(ap.ap[0]), [0, count], list(ap.ap[1])])


def _bcast_inner(ap, count):
    """[P, n] AP -> [P, n, count] with a stride-0 inner axis."""
    return bass.AP(ap.tensor, ap.offset, [list(ap.ap[0]), list(ap.ap[1]), [0, count]])


def _ins(bi):
    return bi.ins if hasattr(bi, "ins") else bi


def build_kernel(tc, x, y):
    nc = tc.nc
    xv = x.rearrange("b t c -> (b t) c")   # [R*T, C] frame rows, offset 0
    yv = y.rearrange("b t c -> (b t) c")

    from contextlib import ExitStack

    with ExitStack() as ctx:
        const = ctx.enter_context(tc.tile_pool(name="const", bufs=1))
        data = ctx.enter_context(tc.tile_pool(name="data", bufs=3))
        det = ctx.enter_context(tc.tile_pool(name="det", bufs=2))
        small = ctx.enter_context(tc.tile_pool(name="small", bufs=1))
        win = ctx.enter_context(tc.tile_pool(name="win", bufs=1))
        psum = ctx.enter_context(tc.tile_pool(name="psum", bufs=1, space="PSUM"))
        dram = ctx.enter_context(tc.tile_pool(name="dram", bufs=1, space="DRAM"))
        # ---- constants (built on device) ----
        ident = const.tile([P, P], F32)
        make_identity(nc, ident[:])
        # I128f[p, j] = 128*p + j  (= row_in_chunk*2048 + t_in_row)
        i128f = const.tile([P, FPP], F32)
        nc.gpsimd.iota(i128f[:], pattern=[[1, FPP]], base=0,
                       channel_multiplier=FPP,
                       allow_small_or_imprecise_dtypes=True)
        # Fgrid[p, f] = f
        fgrid = const.tile([P, WF], F32)
        nc.gpsimd.iota(fgrid[:], pattern=[[1, WF]], base=0,
                       channel_multiplier=0,
                       allow_small_or_imprecise_dtypes=True)
        # C2048[c, i] = 2048 * i   (row-in-chunk offset correction)
        c2048 = const.tile([CHUNKS, RCH], F32)
        nc.gpsimd.iota(c2048[:], pattern=[[T, RCH]], base=0,
                       channel_multiplier=0,
                       allow_small_or_imprecise_dtypes=True)
        # per-partition helpers
        pidx = const.tile([P, 1], I32)
        nc.gpsimd.iota(pidx[:], pattern=[[1, 1]], base=0, channel_multiplier=1)
        idx4 = const.tile([P, 1], I32)          # p // 4  (scratch gather idx)
        nc.vector.tensor_scalar(out=idx4[:], in0=pidx[:], scalar1=2,
                                scalar2=None, op0=OP.arith_shift_right)
        d128i = const.tile([P, 1], I32)
        nc.vector.tensor_scalar(out=d128i[:], in0=idx4[:], scalar1=11,
                                scalar2=None, op0=OP.arith_shift_left)
        d128f = const.tile([P, 1], F32)         # (p//4) * 2048
        nc.vector.tensor_copy(out=d128f[:], in_=d128i[:])
        pm4 = const.tile([P, 1], I32)
        nc.vector.tensor_scalar(out=pm4[:], in0=pidx[:], scalar1=3,
                                scalar2=None, op0=OP.bitwise_and)
        w128i = const.tile([P, 1], I32)
        nc.vector.tensor_scalar(out=w128i[:], in0=pm4[:], scalar1=128, scalar2=1,
                                op0=OP.mult, op1=OP.add)
        w128f = const.tile([P, 1], F32)         # 1 + 128*(p%4)
        nc.vector.tensor_copy(out=w128f[:], in_=w128i[:])

        # ---- bulk copy + per-partition gap detection ----
        m = small.tile([P, 2 * CHUNKS], F32)    # cols 0-3 min-partials, 4-7 neg-max
        stores = []
        for k in range(CHUNKS):
            xk = data.tile([P, FPP * C], F32)
            src = x[k * RCH:(k + 1) * RCH].rearrange(
                "r (q j) c -> (r q) (j c)", q=PPR)
            nc.sync.dma_start(out=xk[:], in_=src)

            samp = xk[:].rearrange("p (j c) -> p j c", c=C)[:, :, 0:1]
            v = det.tile([P, FPP], F32)
            nc.vector.tensor_tensor(
                out=v[:].rearrange("p (j o) -> p j o", o=1),
                in0=samp, in1=samp, op=OP.is_equal)
            bb = det.tile([P, FPP], F32)
            # valid*BIG + t_chunk : min over j = first NaN t (per partition)
            nc.vector.scalar_tensor_tensor(
                out=bb[:], in0=v[:], scalar=BIG, in1=i128f[:],
                op0=OP.mult, op1=OP.add)
            nc.vector.tensor_reduce(out=m[:, k:k + 1], in_=bb[:],
                                    axis=AX.X, op=OP.min)
            dd = det.tile([P, FPP], F32)
            # (valid-1)*t_chunk = -(t on NaN frames) : min over j = -last NaN t
            nc.vector.scalar_tensor_tensor(
                out=dd[:], in0=v[:], scalar=-1.0, in1=i128f[:],
                op0=OP.add, op1=OP.mult)
            nc.vector.tensor_reduce(out=m[:, CHUNKS + k:CHUNKS + k + 1],
                                    in_=dd[:], axis=AX.X, op=OP.min)

            dst = y[k * RCH:(k + 1) * RCH].rearrange(
                "r (q j) c -> (r q) (j c)", q=PPR)
            st = nc.sync.dma_start(out=dst, in_=xk[:])
            stores.append(st)

        # ---- cross-partition reduce to per-row s, e, 1/(e-s) ----
        mt = psum.tile([2 * CHUNKS, P], F32)
        nc.tensor.transpose(out=mt[:], in_=m[:], identity=ident[:])
        mins = small.tile([2 * CHUNKS, RCH], F32)
        nc.vector.tensor_reduce(
            out=mins[:], in_=mt[:].rearrange("p (i w) -> p i w", w=PPR),
            axis=AX.X, op=OP.min)
        # rows 0-3: first_nan + 2048*i ; rows 4-7: -(last_nan + 2048*i)
        n2 = small.tile([CHUNKS, RCH], F32)
        nc.sync.dma_start(out=n2[:], in_=mins[CHUNKS:2 * CHUNKS, :])

        fn4 = mins[0:CHUNKS, :]
        pk = small.tile([CHUNKS, RCH * 4], F32)
        nc.vector.memset(pk[:], 0.0)
        pkv = pk[:].rearrange("c (i k) -> c i k", k=4)
        c2v = c2048[:].rearrange("c (i o) -> c i o", o=1)
        # s = first_nan - 2048*i - 1
        nc.vector.scalar_tensor_tensor(
            out=pkv[:, :, 0:1],
            in0=fn4.rearrange("c (i o) -> c i o", o=1), scalar=-1.0,
            in1=c2v, op0=OP.add, op1=OP.subtract)
        # e = last_nan + 1 = -(n2 + 2048*i) + 1
        t2 = small.tile([CHUNKS, RCH], F32)
        nc.vector.tensor_tensor(
            out=t2[:].rearrange("c (i o) -> c i o", o=1),
            in0=n2[:].rearrange("c (i o) -> c i o", o=1),
            in1=c2v, op=OP.add)
        nc.vector.tensor_scalar(
            out=pkv[:, :, 1:2],
            in0=t2[:].rearrange("c (i o) -> c i o", o=1),
            scalar1=-1.0, scalar2=1.0, op0=OP.mult, op1=OP.add)
        # 1 / (e - s)
        es = small.tile([CHUNKS, RCH], F32)
        nc.vector.tensor_tensor(
            out=es[:].rearrange("c (i o) -> c i o", o=1),
            in0=pkv[:, :, 1:2], in1=pkv[:, :, 0:1], op=OP.subtract)
        nc.vector.reciprocal(
            out=pkv[:, :, 2:3],
            in_=es[:].rearrange("c (i o) -> c i o", o=1))

        scr = dram.tile([R, 4], F32)
        wsc = nc.sync.dma_start(
            out=scr[:].rearrange("(c i) k -> c (i k)", c=CHUNKS), in_=pk[:])

        # ---- replicate per-row scalars to [128, .] via indirect gather ----
        g = small.tile([P, 4], F32)
        gi = nc.gpsimd.indirect_dma_start(
            out=g[:], out_offset=None, in_=scr[:],
            in_offset=bass.IndirectOffsetOnAxis(ap=idx4[:, 0:1], axis=0))
        add_dep_helper(_ins(gi), _ins(wsc), reason="gather scalars after scratch write")

        fxs = small.tile([P, 1], F32)
        nc.vector.tensor_tensor(out=fxs[:], in0=g[:, 0:1], in1=d128f[:], op=OP.add)
        ixs = small.tile([P, 1], I32)
        nc.vector.tensor_copy(out=ixs[:], in_=fxs[:])
        fxe = small.tile([P, 1], F32)
        nc.vector.tensor_tensor(out=fxe[:], in0=g[:, 1:2], in1=d128f[:], op=OP.add)
        ixe = small.tile([P, 1], I32)
        nc.vector.tensor_copy(out=ixe[:], in_=fxe[:])
        fww = small.tile([P, 1], F32)
        nc.vector.tensor_tensor(out=fww[:], in0=fxs[:], in1=w128f[:], op=OP.add)
        ixw = small.tile([P, 1], I32)
        nc.vector.tensor_copy(out=ixw[:], in_=fww[:])

        xs = small.tile([P, C], F32)
        nc.gpsimd.indirect_dma_start(
            out=xs[:], out_offset=None, in_=xv,
            in_offset=bass.IndirectOffsetOnAxis(ap=ixs[:, 0:1], axis=0))
        xe = small.tile([P, C], F32)
        nc.gpsimd.indirect_dma_start(
            out=xe[:], out_offset=None, in_=xv,
            in_offset=bass.IndirectOffsetOnAxis(ap=ixe[:, 0:1], axis=0))

        df = small.tile([P, C], F32)
        nc.vector.tensor_tensor(out=df[:], in0=xe[:], in1=xs[:], op=OP.subtract)
        slope = small.tile([P, C], F32)
        nc.vector.tensor_scalar(out=slope[:], in0=df[:], scalar1=g[:, 2:3],
                                scalar2=None, op0=OP.mult)
        base = small.tile([P, C], F32)
        # base = xs + (1 + 128*(p%4)) * slope
        nc.vector.scalar_tensor_tensor(
            out=base[:], in0=slope[:], scalar=w128f[:, 0:1], in1=xs[:],
            op0=OP.mult, op1=OP.add)

        # ---- window gather, interp, select, scatter ----
        xw = win.tile([P, WF * C], F32)
        nc.gpsimd.indirect_dma_start(
            out=xw[:], out_offset=None, in_=xv,
            in_offset=bass.IndirectOffsetOnAxis(ap=ixw[:, 0:1], axis=0))

        prod = win.tile([P, WF * C], F32)
        prod3 = prod[:].rearrange("p (f c) -> p f c", c=C)
        nc.vector.tensor_tensor(
            out=prod3, in0=_bcast_inner(fgrid[:], C),
            in1=_bcast_mid(slope[:], WF), op=OP.mult)
        nc.vector.tensor_tensor(
            out=prod3, in0=prod3, in1=_bcast_mid(base[:], WF), op=OP.add)
        vm = win.tile([P, WF * C], mybir.dt.uint8)
        nc.vector.tensor_tensor(out=vm[:], in0=xw[:], in1=xw[:], op=OP.is_equal)
        nc.vector.copy_predicated(out=prod[:], mask=vm[:], data=xw[:])

        sc = nc.gpsimd.indirect_dma_start(
            out=yv, out_offset=bass.IndirectOffsetOnAxis(ap=ixw[:, 0:1], axis=0),
            in_=prod[:], in_offset=None)
        for st in stores:
            add_dep_helper(_ins(sc), _ins(st), reason="scatter windows after bulk store")


_NC = None


def _get_nc():
    global _NC
    if _NC is None:
        nc = bacc.Bacc("TRN2", target_bir_lowering=False, debug=False,
                       num_devices=NCORES)
        x = nc.dram_tensor("x", [R, T, C], F32, kind="ExternalInput")
        y = nc.dram_tensor("y", [R, T, C], F32, kind="ExternalOutput")
        with tile.TileContext(nc) as tc:
            build_kernel(tc, x.ap(), y.ap())
        nc.compile()
        _NC = nc
    return _NC


def kernel(x):
    from concourse.bass_utils import run_bass_kernel_spmd

    x = np.ascontiguousarray(x, dtype=np.float32)
    assert x.shape == (B, T, A, D), x.shape
    xr = x.reshape(NCORES, R, T, C)
    nc = _get_nc()
    in_maps = [{"x": xr[i]} for i in range(NCORES)]
    res = run_bass_kernel_spmd(nc, in_maps, core_ids=list(range(NCORES)))
    out = np.stack([res.results[i]["y"] for i in range(NCORES)])
    return out.reshape(B, T, A, D)



# revision 30
# speedup vs baseline: 1.1990x; 1.1990x over previous
"""Trainium2 Bass kernel for nn_InterpolationModel (NaN-gap linear interpolation).

Problem: x [256, 2048, 22, 2] f32, one contiguous NaN gap along T per batch row.
Output: x with the gap filled by linear interpolation between the last valid
frame before the gap (s) and the first valid frame after it (e).

Strategy (pure data parallel over batch, 32 rows per core, 4 chunks of 8 rows,
everything pipelined per chunk so DMA stays saturated):
  - Bulk copy x -> y through SBUF ([128, 5632] tiles, partition = 128
    consecutive frames, 22.5KB contiguous per partition).
  - While a chunk transits SBUF, sample element 0 of every frame and reduce
    per partition: min(t + 65536*valid) and min((valid-1)*t) -> first/last
    NaN frame per partition. Two small TensorE transposes + grouped reduce
    turn the partials into per-row gs (global frame of last valid before gap),
    ge (first valid after), 1/(e-s) living on 8 partitions.
  - Tiny indirect gathers fetch the two endpoint frames x[gs], x[ge] ([8,44]
    each); one selection matmul (sel8[r,p] = (p%8 == r)) broadcasts scalars +
    endpoints to all 128 partitions. No DRAM scratch round trip.
  - Interp values never depend on the gap contents, so no window gather:
    partition p (row r=p%8, slice w=p//8) computes 32 frames of interp at
    posB = clamp(gs+1+32w, gs+1, ge-32) and scatters them to y (overlapping
    slices stack on identical values; scatter-B depends only on this chunk's
    bulk store).
  - A second [128, 88] pass covers [gs+1, gs+32] in 2-frame slices with a
    gather + NaN-mask blend (handles gap < 32 rows where scatter-B's clamped
    slices wrote interp over valid frames); scatter-A is ordered after
    scatter-B so it always lands last.
"""

import numpy as np

import concourse.bacc as bacc
import concourse.bass as bass
import concourse.mybir as mybir
import concourse.tile as tile
from bass_rust import add_dep_helper
from concourse.masks import make_identity

F32 = mybir.dt.float32
I32 = mybir.dt.int32
U8 = mybir.dt.uint8
AX = mybir.AxisListType
OP = mybir.AluOpType

# Full problem: B=256, T=2048, A=22, D=2 over 8 cores.
B, T, A, D = 256, 2048, 22, 2
C = A * D            # 44 contiguous f32 per frame
NCORES = 8
R = B // NCORES      # 32 rows per core
CHUNKS = 2           # bulk-copy halves per core
RCH = R // CHUNKS    # 16 rows per half
P = 128
FPP = (RCH * T) // P  # 256 frames per partition in a half tile
PPR = T // FPP        # 8 partitions per row
WF = 64               # frames per scatter-B slice (8 slices * 64 = 512 cover)
AF = 8                # frames per blend-A slice (8 slices * 8 = 64 window)
BIG = 65536.0


def _bcast_mid(ap, count):
    """[P, n] AP -> [P, count, n] with a stride-0 middle axis."""
    return bass.AP(ap.tensor, ap.offset, [list(ap.ap[0]), [0, count], list(ap.ap[1])])


def _bcast_inner(ap, count):
    """[P, n] AP -> [P, n, count] with a stride-0 inner axis."""
    return bass.AP(ap.tensor, ap.offset, [list(ap.ap[0]), list(ap.ap[1]), [0, count]])


def _ins(bi):
    return bi.ins if hasattr(bi, "ins") else bi


def _axes(ap, axes):
    """AP with the same tensor/offset but custom axis list."""
    return bass.AP(ap.tensor, ap.offset, [list(a) for a in axes])


def build_kernel(tc, x, ys):
    nc = tc.nc
    xv = x.rearrange("b t c -> (b t) c")   # [R*T, C] frame rows, offset 0
    K = CHUNKS
    NR = R                                 # 32 rows per core
    NB = 3 + 2 * C                         # rhs32 row: gs, ge, inv, x[gs], x[ge]
    HOFF = RCH * T                         # 32768 frame rows per half

    from contextlib import ExitStack

    with ExitStack() as ctx:
        const = ctx.enter_context(tc.tile_pool(name="const", bufs=1))
        data = ctx.enter_context(tc.tile_pool(name="data", bufs=1))
        det = ctx.enter_context(tc.tile_pool(name="det", bufs=1))
        small = ctx.enter_context(tc.tile_pool(name="small", bufs=1))
        win = ctx.enter_context(tc.tile_pool(name="win", bufs=1))
        psum = ctx.enter_context(tc.tile_pool(name="psum", bufs=1, space="PSUM"))

        # ---- constants (built on device, off the critical path) ----
        ident = const.tile([P, P], F32)
        make_identity(nc, ident[:])
        # I128f[p, j] = FPP*p + j  (= row_in_half*2048 + t_in_row)
        i128f = const.tile([P, FPP], F32)
        nc.gpsimd.iota(i128f[:], pattern=[[1, FPP]], base=0,
                       channel_multiplier=FPP,
                       allow_small_or_imprecise_dtypes=True)
        # fgridB[p, f] = f for f < WF ; fgridA[p, f] = f for f < AF
        fgridB = const.tile([P, WF], F32)
        nc.gpsimd.iota(fgridB[:], pattern=[[1, WF]], base=0,
                       channel_multiplier=0,
                       allow_small_or_imprecise_dtypes=True)
        fgridA = const.tile([P, AF], F32)
        nc.gpsimd.iota(fgridA[:], pattern=[[1, AF]], base=0,
                       channel_multiplier=0,
                       allow_small_or_imprecise_dtypes=True)
        # per-partition slice helpers: row r = p % 16, slice w = p // 16
        pidx = const.tile([P, 1], I32)
        nc.gpsimd.iota(pidx[:], pattern=[[1, 1]], base=0, channel_multiplier=1)
        pdiv16 = const.tile([P, 1], I32)
        nc.vector.tensor_scalar(out=pdiv16[:], in0=pidx[:], scalar1=4,
                                scalar2=None, op0=OP.arith_shift_right)
        wBi = const.tile([P, 1], I32)
        nc.vector.tensor_scalar(out=wBi[:], in0=pdiv16[:], scalar1=WF, scalar2=1,
                                op0=OP.mult, op1=OP.add)
        wBp1 = const.tile([P, 1], F32)       # 1 + WF*(p//16)
        nc.vector.tensor_copy(out=wBp1[:], in_=wBi[:])
        wAi = const.tile([P, 1], I32)
        nc.vector.tensor_scalar(out=wAi[:], in0=pdiv16[:], scalar1=AF, scalar2=1,
                                op0=OP.mult, op1=OP.add)
        wAp1 = const.tile([P, 1], F32)       # 1 + AF*(p//16)
        nc.vector.tensor_copy(out=wAp1[:], in_=wAi[:])
        pd16f = const.tile([P, 1], F32)
        nc.vector.tensor_copy(out=pd16f[:], in_=pdiv16[:])
        w0h = const.tile([P, 1], F32)        # HUGE on slice-0 partitions
        nc.vector.tensor_scalar(out=w0h[:], in0=pd16f[:], scalar1=0.0,
                                scalar2=1048576.0, op0=OP.is_equal, op1=OP.mult)
        # [NR, .] row-index helpers: global row i = RCH*h + r
        i32r = const.tile([NR, 1], I32)
        nc.gpsimd.iota(i32r[:], pattern=[[1, 1]], base=0, channel_multiplier=1)
        cbase = const.tile([NR, 1], I32)      # HOFF * (i // RCH)
        nc.vector.tensor_scalar(out=cbase[:], in0=i32r[:], scalar1=4,
                                scalar2=15, op0=OP.arith_shift_right,
                                op1=OP.arith_shift_left)
        cbasef = const.tile([NR, 1], F32)
        nc.vector.tensor_copy(out=cbasef[:], in_=cbase[:])
        cbm1 = const.tile([NR, 1], F32)       # HOFF*(i//RCH) - 1
        nc.vector.tensor_scalar(out=cbm1[:], in0=cbasef[:], scalar1=-1.0,
                                scalar2=None, op0=OP.add)
        cbp1 = const.tile([NR, 1], F32)       # HOFF*(i//RCH) + 1
        nc.vector.tensor_scalar(out=cbp1[:], in0=cbasef[:], scalar1=1.0,
                                scalar2=None, op0=OP.add)
        # sel_h[i, p] = 1.0 if i == RCH*h + p % 16
        im32 = const.tile([NR, P], I32)
        nc.gpsimd.iota(im32[:], pattern=[[1, P]], base=0, channel_multiplier=0)
        im32m = const.tile([NR, P], I32)
        nc.vector.tensor_scalar(out=im32m[:], in0=im32[:], scalar1=RCH - 1,
                                scalar2=None, op0=OP.bitwise_and)
        im32f = const.tile([NR, P], F32)
        nc.vector.tensor_copy(out=im32f[:], in_=im32m[:])
        rowf = const.tile([NR, 1], F32)
        nc.vector.tensor_copy(out=rowf[:], in_=i32r[:])
        sel = []
        for k in range(K):
            rk = const.tile([NR, 1], F32, name=f"rk{k}")
            nc.vector.tensor_scalar(out=rk[:], in0=rowf[:],
                                    scalar1=float(-RCH * k), scalar2=None,
                                    op0=OP.add)
            sk = const.tile([NR, P], F32, name=f"sel{k}")
            nc.vector.tensor_scalar(out=sk[:], in0=im32f[:], scalar1=rk[:, 0:1],
                                    scalar2=None, op0=OP.is_equal)
            sel.append(sk)
        # chalf[p, k] = HOFF * k (half-local offset correction)
        chalf0 = const.tile([P, K], F32)
        nc.gpsimd.iota(chalf0[:], pattern=[[1, K]], base=0,
                       channel_multiplier=0,
                       allow_small_or_imprecise_dtypes=True)
        chalf = const.tile([P, K], F32)
        nc.vector.tensor_scalar(out=chalf[:], in0=chalf0[:],
                                scalar1=float(HOFF), scalar2=None, op0=OP.mult)

        # ---- all bulk loads, then all bulk stores (sync engine order) ----
        xk = [None] * K
        st = [None] * K
        for k in range(K):
            xk[k] = data.tile([P, FPP * C], F32, name=f"xk{k}")
            src = x[k * RCH:(k + 1) * RCH].rearrange(
                "r (q j) c -> (r q) (j c)", q=PPR)
            nc.sync.dma_start(out=xk[k][:], in_=src)
        for k in range(K):
            dst = ys[k].rearrange("r (q j) c -> (r q) (j c)", q=PPR)
            st[k] = nc.sync.dma_start(out=dst, in_=xk[k][:])

        # ---- stage V1: per-partition gap detection (gated on each load) ----
        m = [None] * K
        for k in range(K):
            samp = xk[k][:].rearrange("p (j c) -> p j c", c=C)[:, :, 0:1]
            v = det.tile([P, FPP], F32, name=f"v{k}")
            nc.vector.tensor_tensor(
                out=v[:].rearrange("p (j o) -> p j o", o=1),
                in0=samp, in1=samp, op=OP.is_equal)
            m[k] = det.tile([P, 2], F32, name=f"m{k}")
            bb = det.tile([P, FPP], F32, name=f"bb{k}")
            # valid*BIG + idx : min over j = first NaN idx (idx = 2048r + t)
            nc.vector.scalar_tensor_tensor(
                out=bb[:], in0=v[:], scalar=BIG, in1=i128f[:],
                op0=OP.mult, op1=OP.add)
            nc.vector.tensor_reduce(out=m[k][:, 0:1], in_=bb[:], axis=AX.X,
                                    op=OP.min)
            dd = det.tile([P, FPP], F32, name=f"dd{k}")
            # (valid-1)*idx : min over j = -(last NaN idx)
            nc.vector.scalar_tensor_tensor(
                out=dd[:], in0=v[:], scalar=-1.0, in1=i128f[:],
                op0=OP.add, op1=OP.mult)
            nc.vector.tensor_reduce(out=m[k][:, 1:2], in_=dd[:], axis=AX.X,
                                    op=OP.min)

        # ---- P1: transpose all partials into one PSUM tile ----
        mtall = psum.tile([2, P * K], F32, name="mtall")
        for k in range(K):
            nc.tensor.transpose(out=mtall[:, k * P:(k + 1) * P], in_=m[k][:],
                                identity=ident[:])

        # ---- V2 + P2: one grouped reduce, one transpose -> [NR, 2] ----
        minsall = small.tile([2, NR], F32)
        nc.vector.tensor_reduce(
            out=minsall[:], in_=mtall[:].rearrange("p (i w) -> p i w", w=PPR),
            axis=AX.X, op=OP.min)
        fgall = psum.tile([NR, 2], F32, name="fgall")
        nc.tensor.transpose(out=fgall[:], in_=minsall[:],
                            identity=ident[0:2, 0:2])

        # ---- V3: packed per-row global scalars on NR partitions ----
        # rhs32 cols: 0=gs, 1=ge, 2=1/(e-s), 3:47=x[gs], 47:91=x[ge]
        rhs32 = small.tile([NR, NB], F32)
        nc.vector.tensor_tensor(out=rhs32[:, 0:1], in0=fgall[:, 0:1],
                                in1=cbm1[:], op=OP.add)
        nc.vector.scalar_tensor_tensor(
            out=rhs32[:, 1:2], in0=fgall[:, 1:2], scalar=-1.0, in1=cbp1[:],
            op0=OP.mult, op1=OP.add)
        es32 = small.tile([NR, 1], F32)
        nc.vector.tensor_tensor(out=es32[:], in0=rhs32[:, 1:2],
                                in1=rhs32[:, 0:1], op=OP.subtract)
        nc.vector.reciprocal(out=rhs32[:, 2:3], in_=es32[:])
        ixs32 = small.tile([NR, 1], I32)
        nc.vector.tensor_copy(out=ixs32[:], in_=rhs32[:, 0:1])
        ixe32 = small.tile([NR, 1], I32)
        nc.vector.tensor_copy(out=ixe32[:], in_=rhs32[:, 1:2])

        # ---- G1: endpoint frame gathers (one call each) ----
        nc.gpsimd.indirect_dma_start(
            out=rhs32[:, 3:3 + C], out_offset=None, in_=xv,
            in_offset=bass.IndirectOffsetOnAxis(ap=ixs32[:, 0:1], axis=0))
        nc.gpsimd.indirect_dma_start(
            out=rhs32[:, 3 + C:NB], out_offset=None, in_=xv,
            in_offset=bass.IndirectOffsetOnAxis(ap=ixe32[:, 0:1], axis=0))

        # ---- P3: broadcast row scalars + endpoints to 128 partitions ----
        bcall = psum.tile([P, NB * K], F32, name="bcall")
        for k in range(K):
            nc.tensor.matmul(bcall[:, NB * k:NB * (k + 1)], lhsT=sel[k][:],
                             rhs=rhs32[:], start=True, stop=True)

        # strided views over bcall: [p, k, .] with k-stride NB
        b0 = bcall[:, 0:1]
        pax = list(b0.ap[0])
        gs_v = _axes(b0, [pax, [NB, K], [1, 1]])
        ge_v = _axes(bcall[:, 1:2], [pax, [NB, K], [1, 1]])
        inv_v44 = _axes(bcall[:, 2:3], [pax, [NB, K], [0, C]])
        xe_v = _axes(bcall[:, 3 + C:NB], [pax, [NB, K], [1, C]])

        # ---- V4: packed [128, K] offset math + [128, K*44] coefficients ----
        def v3d(t):
            return t[:].rearrange("p (k o) -> p k o", o=1)

        t1 = win.tile([P, K], F32)
        nc.vector.tensor_tensor(out=v3d(t1), in0=gs_v,
                                in1=_bcast_mid(wBp1[:], K), op=OP.add)
        posB = win.tile([P, K], F32)
        # posB = max(min(gs+1+WF*w, ge-WF), gs+1)
        nc.vector.scalar_tensor_tensor(
            out=v3d(posB), in0=ge_v, scalar=float(-WF), in1=v3d(t1),
            op0=OP.add, op1=OP.min)
        nc.vector.scalar_tensor_tensor(
            out=v3d(posB), in0=gs_v, scalar=1.0, in1=v3d(posB),
            op0=OP.add, op1=OP.max)
        posBl = win.tile([P, K], F32)
        nc.vector.tensor_tensor(out=posBl[:], in0=posB[:], in1=chalf[:],
                                op=OP.subtract)
        # skip slices whose span is not needed (WF*w >= g <=> t1 >= ge),
        # and slice 0 always (the blend pass covers [gs+1, gs+WF])
        nd = win.tile([P, K], F32)
        nc.vector.tensor_tensor(out=v3d(nd), in0=v3d(t1), in1=ge_v,
                                op=OP.is_lt)
        skadd = win.tile([P, K], F32)
        nc.vector.tensor_scalar(out=skadd[:], in0=nd[:], scalar1=-1.0,
                                scalar2=-1048576.0, op0=OP.add, op1=OP.mult)
        nc.vector.tensor_tensor(out=posBl[:], in0=posBl[:], in1=skadd[:],
                                op=OP.add)
        nc.vector.tensor_tensor(out=v3d(posBl), in0=v3d(posBl),
                                in1=_bcast_mid(w0h[:], K), op=OP.add)
        ioffB = win.tile([P, K], I32)
        nc.vector.tensor_copy(out=ioffB[:], in_=posBl[:])
        deltaB = win.tile([P, K], F32)
        nc.vector.tensor_tensor(out=v3d(deltaB), in0=v3d(posB), in1=gs_v,
                                op=OP.subtract)
        fA = win.tile([P, K], F32)
        nc.vector.tensor_tensor(out=v3d(fA), in0=gs_v,
                                in1=_bcast_mid(wAp1[:], K), op=OP.add)
        iposA = win.tile([P, K], I32)
        nc.vector.tensor_copy(out=iposA[:], in_=fA[:])
        fAl = win.tile([P, K], F32)
        nc.vector.tensor_tensor(out=fAl[:], in0=fA[:], in1=chalf[:],
                                op=OP.subtract)
        iposAl = win.tile([P, K], I32)
        nc.vector.tensor_copy(out=iposAl[:], in_=fAl[:])
        xsall = win.tile([P, K * C], F32)
        xs3 = xsall[:].rearrange("p (k c) -> p k c", c=C)
        nc.vector.tensor_copy(out=xs3, in_=_axes(bcall[:, 3:3 + C],
                                                 [pax, [NB, K], [1, C]]))
        dfall = win.tile([P, K * C], F32)
        df3 = dfall[:].rearrange("p (k c) -> p k c", c=C)
        nc.vector.tensor_tensor(out=df3, in0=xe_v, in1=xs3, op=OP.subtract)
        slopeall = win.tile([P, K * C], F32)
        sl3 = slopeall[:].rearrange("p (k c) -> p k c", c=C)
        nc.vector.tensor_tensor(out=sl3, in0=df3, in1=inv_v44, op=OP.mult)
        baseBall = win.tile([P, K * C], F32)
        bB3 = baseBall[:].rearrange("p (k c) -> p k c", c=C)
        nc.vector.tensor_tensor(out=bB3, in0=sl3,
                                in1=_bcast_inner(deltaB[:], C), op=OP.mult)
        nc.vector.tensor_tensor(out=bB3, in0=bB3, in1=xs3, op=OP.add)
        baseAall = win.tile([P, K * C], F32)
        bA3 = baseAall[:].rearrange("p (k c) -> p k c", c=C)
        nc.vector.scalar_tensor_tensor(
            out=bA3, in0=sl3, scalar=wAp1[:, 0:1], in1=xs3,
            op0=OP.mult, op1=OP.add)

        # ---- G2: blend-window gathers (one [128,1]-offset call per half) ----
        AW = AF * C
        xw88 = win.tile([P, K * AW], F32)
        for k in range(K):
            nc.gpsimd.indirect_dma_start(
                out=xw88[:, AW * k:AW * (k + 1)], out_offset=None, in_=xv,
                in_offset=bass.IndirectOffsetOnAxis(ap=iposA[:, k:k + 1],
                                                    axis=0))

        # ---- V5a: prodA packed ----
        prodA = win.tile([P, K * AW], F32)
        for k in range(K):
            pa3 = prodA[:, AW * k:AW * (k + 1)].rearrange(
                "p (f c) -> p f c", c=C)
            slk = slopeall[:, C * k:C * (k + 1)]
            nc.vector.tensor_tensor(
                out=pa3, in0=_bcast_inner(fgridA[:], C),
                in1=_bcast_mid(slk, AF), op=OP.mult)
            nc.vector.tensor_tensor(
                out=pa3, in0=pa3,
                in1=_bcast_mid(baseAall[:, C * k:C * (k + 1)], AF), op=OP.add)

        # ---- V5b + G3: per half, big interp products then scatters ----
        prodB = [None] * K

        def emit_prodB(k):
            prodB[k] = win.tile([P, WF * C], F32, name=f"prodB{k}")
            p3 = prodB[k][:].rearrange("p (f c) -> p f c", c=C)
            slk = slopeall[:, C * k:C * (k + 1)]
            nc.vector.tensor_tensor(
                out=p3, in0=_bcast_inner(fgridB[:], C),
                in1=_bcast_mid(slk, WF), op=OP.mult)
            nc.vector.tensor_tensor(
                out=p3, in0=p3, in1=_bcast_mid(baseBall[:, C * k:C * (k + 1)], WF),
                op=OP.add)

        def emit_scB(k):
            yvk = ys[k].rearrange("r t c -> (r t) c")   # [RCH*T, C], offset 0
            scB = nc.gpsimd.indirect_dma_start(
                out=yvk,
                out_offset=bass.IndirectOffsetOnAxis(ap=ioffB[:, k:k + 1],
                                                     axis=0),
                in_=prodB[k][:], in_offset=None,
                bounds_check=RCH * T - 1, oob_is_err=False)
            add_dep_helper(_ins(scB), _ins(st[k]),
                           reason="scatter B after bulk store")

        def emit_scA(k):
            yvk = ys[k].rearrange("r t c -> (r t) c")
            scA = nc.gpsimd.indirect_dma_start(
                out=yvk,
                out_offset=bass.IndirectOffsetOnAxis(ap=iposAl[:, k:k + 1],
                                                     axis=0),
                in_=prodA[:, AW * k:AW * (k + 1)], in_offset=None)
            add_dep_helper(_ins(scA), _ins(st[k]),
                           reason="scatter A after bulk store")

        emit_prodB(0)
        emit_scB(0)
        vmA = win.tile([P, K * AW], U8)
        nc.vector.tensor_tensor(out=vmA[:], in0=xw88[:], in1=xw88[:],
                                op=OP.is_equal)
        nc.vector.copy_predicated(out=prodA[:], mask=vmA[:], data=xw88[:])
        emit_scA(0)
        emit_scA(1)
        emit_prodB(1)
        emit_scB(1)


_NC = None


def _get_nc():
    global _NC
    if _NC is None:
        nc = bacc.Bacc("TRN2", target_bir_lowering=False, debug=False,
                       num_devices=NCORES)
        x = nc.dram_tensor("x", [R, T, C], F32, kind="ExternalInput")
        ys = [nc.dram_tensor(f"y{k}", [RCH, T, C], F32, kind="ExternalOutput")
              for k in range(CHUNKS)]
        with tile.TileContext(nc) as tc:
            build_kernel(tc, x.ap(), [yk.ap() for yk in ys])
        nc.compile()
        _NC = nc
    return _NC


def kernel(x):
    from concourse.bass_utils import run_bass_kernel_spmd

    x = np.ascontiguousarray(x, dtype=np.float32)
    assert x.shape == (B, T, A, D), x.shape
    xr = x.reshape(NCORES, R, T, C)
    nc = _get_nc()
    in_maps = [{"x": xr[i]} for i in range(NCORES)]
    res = run_bass_kernel_spmd(nc, in_maps, core_ids=list(range(NCORES)))
    out = np.stack([
        np.concatenate([res.results[i][f"y{k}"] for k in range(CHUNKS)], axis=0)
        for i in range(NCORES)
    ])
    return out.reshape(B, T, A, D)
